# revision 7
# baseline (speedup 1.0000x reference)
"""Bass/Trainium2 kernel for the 2-layer GAT model (nn_GATModel).

V2 design (8-core SPMD, edge parallelism, dma_gather-based):
  Host: balance nodes into 432 blocks of 128 slots (greedy by in-degree) so
  every block has <= CM*128 in-edges and each src-half (table rows split at
  NPAD/2 for int16 dma_gather indices) has <= HALFCAP edges. Edges of a block
  live in slots [0, HALFCAP) (src in lo half) and [HALFCAP, 2*HALFCAP)
  (src in hi half); per-chunk one-hot G [lane, dstslot] and GT [dstslot,
  lane] are fp8.
  Device, per core:
    Phase Z (replicated): z|el|er = x @ [W1|W1@Al|W1@Ar] -> split tables
      z_lo/z_hi [NPAD/2, 384] bf16 (rows [z(256)|el(8)|er(8)|pad]); er also
      written to a block-transposed compact table er_tblT [(p, gb), 8].
    E1: er_own [128, NB, 8] via ONE indirect gather (per-core row offset
      data). Per block: two dma_gathers (lo/hi windows, 1024 idx each) fetch
      zsel [128, 16, 384]; erd = GT_chunk^T @ er_own (fp8 x bf16 matmuls);
      ex = exp(leakyrelu(el_s + er_d)); rhs = [z*ex | ex]; 16 matmuls
      G_chunk^T @ rhs accumulate [rst|denom]; normalize + b1 + ELU -> h1;
      z2aug = h1 @ W2aug via PE transposes -> z2rows [128, NB, 4] (SBUF).
    AllGather t2loc [NCORE,4] -> t2 [NPAD,4]; expand into padded gather
      tables t2p_lo/hi [NPAD/2, 64] f32.
    E2: per block: two dma_gathers fetch t2sel [128, 16, 64] f32 (cols
      [m0,m1,el2,er2]); er2d = GT_chunk^T @ er2_own (er2_own = z2rows col 3,
      local); ex2 = exp(leakyrelu(el2_s + er2_d)); lv = [m*ex2 | ex2] bf16;
      16 matmuls G_chunk^T @ lv accumulate node-major [rst2|den2]; normalize
      + b2 -> out.
"""

import sys, os, time, heapq
sys.path.insert(0, "/opt/trn_rl_repo")

import numpy as np
from dataclasses import dataclass

from concourse import bass, bacc, mybir, tile
from concourse import bass_utils

P = 128


@dataclass
class Cfg:
    N: int = 50000
    E: int = 800000
    IN: int = 128
    H: int = 8
    F1: int = 32
    C: int = 2
    neg_slope: float = 0.2
    cores: int = 8
    NB: int = 54              # node blocks per core
    CM: int = 16              # chunks (of 128 edges) per block
    body_reps: int = 1
    debug_outs: bool = False

    @property
    def HF(self):
        return self.H * self.F1         # 256

    @property
    def ZW(self):
        return self.HF + 2 * self.H     # 272 (z | el | er)

    @property
    def FW(self):
        return 384                      # padded z row (bf16, 768B)

    @property
    def NBT(self):
        return self.cores * self.NB     # 432 global blocks

    @property
    def NPAD(self):
        return self.NBT * P             # 55296

    @property
    def HALF(self):
        return self.NPAD // 2           # 27648 (int16-safe table half)

    @property
    def NCORE(self):
        return self.NB * P              # 6912

    @property
    def CMH(self):
        return self.CM // 2             # chunks per half-window (8)

    @property
    def WCAP(self):
        return self.CMH * P             # 1024 edges per half-window

    @property
    def NCH(self):
        return self.NB * self.CM        # chunks per core (864)


# ----------------------------------------------------------------------------
# Host-side preprocessing
# ----------------------------------------------------------------------------

def balance_nodes(dst, cfg: Cfg):
    """Assign each node to a (block, slot) so per-block in-degree is
    balanced. Returns pos_of_node [N] (table position)."""
    deg = np.bincount(dst, minlength=cfg.N)
    order = np.argsort(-deg, kind="stable")
    nbt = cfg.NBT
    # heap of (edge_load, block); track slot counts
    heap = [(0, b) for b in range(nbt)]
    heapq.heapify(heap)
    slots_used = np.zeros(nbt, np.int64)
    pos = np.empty(cfg.N, np.int64)
    for node in order:
        while True:
            load, b = heapq.heappop(heap)
            if slots_used[b] < P:
                break
        pos[node] = b * P + slots_used[b]
        slots_used[b] += 1
        heapq.heappush(heap, (load + int(deg[node]), b))
    return pos


def build_host_data(inputs: dict, cfg: Cfg):
    x = np.asarray(inputs["x"], np.float32)
    src = np.asarray(inputs["src"], np.int64)
    dst = np.asarray(inputs["dst"], np.int64)
    W1 = np.asarray(inputs["W1"], np.float32)
    al1 = np.asarray(inputs["attn_l1"], np.float32)
    ar1 = np.asarray(inputs["attn_r1"], np.float32)
    b1 = np.asarray(inputs["b1"], np.float32)
    W2 = np.asarray(inputs["W2"], np.float32)
    al2 = np.asarray(inputs["attn_l2"], np.float32)
    ar2 = np.asarray(inputs["attn_r2"], np.float32)
    b2 = np.asarray(inputs["b2"], np.float32)
    import ml_dtypes
    bf16 = ml_dtypes.bfloat16
    fp8 = ml_dtypes.float8_e4m3

    H, F1, C, HF = cfg.H, cfg.F1, cfg.C, cfg.HF
    NB, CM, CMH, WCAP = cfg.NB, cfg.CM, cfg.CMH, cfg.WCAP
    HALF, NPAD, NCH = cfg.HALF, cfg.NPAD, cfg.NCH

    pos = balance_nodes(dst, cfg)

    # augmented weights [IN, 272]
    Al = np.zeros((HF, H), np.float32)
    Ar = np.zeros((HF, H), np.float32)
    for h in range(H):
        Al[h * F1:(h + 1) * F1, h] = al1[h]
        Ar[h * F1:(h + 1) * F1, h] = ar1[h]
    Waug = np.concatenate([W1, W1 @ Al, W1 @ Ar], axis=1).astype(bf16)
    W2aug = np.concatenate([W2, W2 @ al2.reshape(C, 1),
                            W2 @ ar2.reshape(C, 1)], axis=1).astype(np.float32)

    # permuted xT [128, NPAD] bf16
    xT = np.zeros((cfg.IN, NPAD), bf16)
    xT[:, pos] = x.T.astype(bf16)

    ps = pos[src]
    pd = pos[dst]
    gb = pd // P                 # global block of the edge's dst
    dstslot = pd % P
    is_hi = ps >= HALF

    # per (global block, half) edge lists -> slot assignment
    # slot j in [0, WCAP) for lo, [WCAP, 2*WCAP) for hi
    okey = gb * 2 + is_hi.astype(np.int64)
    eorder = np.argsort(okey, kind="stable")
    s_ps, s_slot_of = ps[eorder], None
    s_dstslot = dstslot[eorder]
    s_okey = okey[eorder]
    starts = np.searchsorted(s_okey, np.arange(cfg.NBT * 2))
    ends = np.searchsorted(s_okey, np.arange(cfg.NBT * 2) + 1)
    counts = ends - starts
    n_lo = counts[0::2]
    n_hi = counts[1::2]
    assert n_lo.max() <= WCAP, f"lo overflow {n_lo.max()}"
    assert n_hi.max() <= WCAP, f"hi overflow {n_hi.max()}"

    in_maps = []
    for c in range(cfg.cores):
        # edge slot arrays per block: [NB, CM*P]
        e_src = np.zeros((NB, CM * P), np.int64)     # table position of src
        e_valid = np.zeros((NB, CM * P), bool)
        e_dslot = np.zeros((NB, CM * P), np.int64)
        for b in range(NB):
            g = c * NB + b
            for half in (0, 1):
                s, e = starts[g * 2 + half], ends[g * 2 + half]
                n = e - s
                off = half * WCAP
                e_src[b, off:off + n] = s_ps[s:e]
                e_dslot[b, off:off + n] = s_dstslot[s:e]
                e_valid[b, off:off + n] = True

        # gather indices (int16): window w = b*2+half covers slots
        # [half*WCAP, (half+1)*WCAP); idx j -> wrapped [j%16, w*64 + j//16]
        idx_val = e_src.reshape(NB, 2, WCAP).copy()
        idx_val[:, 1, :] -= HALF
        idx_val = np.where(e_valid.reshape(NB, 2, WCAP), idx_val, 0)
        flat = idx_val.reshape(NB * 2, WCAP)         # [windows, 1024]
        wrapped = np.zeros((16, NB * 2 * (WCAP // 16)), np.int16)
        j = np.arange(WCAP)
        for w in range(NB * 2):
            wrapped[j % 16, w * (WCAP // 16) + j // 16] = flat[w]
        sidx16 = np.tile(wrapped, (8, 1))            # [128, NB*2*64]

        # one-hot G / GT fp8: [128, NCH*128]
        # chunk ch = b*CM + (slot//128); lane = slot % 128
        lane = np.tile(np.arange(CM * P) % P, (NB, 1))
        ch = (np.arange(CM * P) // P)[None, :] + (np.arange(NB) * CM)[:, None]
        v = e_valid
        G = np.zeros((P, NCH, P), fp8)
        G[lane[v], ch[v], e_dslot[v]] = 1.0
        GT = np.zeros((P, NCH, P), fp8)
        GT[e_dslot[v], ch[v], lane[v]] = 1.0

        m = {
            "xT": xT,
            "Waug": np.ascontiguousarray(Waug),
            "W2aug": W2aug,
            "b1t": np.broadcast_to(b1, (P, HF)).copy(),
            "b2t": np.broadcast_to(b2, (P, C)).copy(),
            "sidx": np.ascontiguousarray(sidx16),
            "G8": np.ascontiguousarray(G.reshape(P, NCH * P)),
            "GT8": np.ascontiguousarray(GT.reshape(P, NCH * P)),
            "nid0": (np.arange(P, dtype=np.int32)[:, None] * cfg.NBT
                     + c * NB + np.arange(NB, dtype=np.int32)[None, :]),
        }
        in_maps.append(m)

    meta = {"pos": pos}
    return in_maps, meta


# ----------------------------------------------------------------------------
# Device program
# ----------------------------------------------------------------------------

def build_program(cfg: Cfg, debug: bool = False) -> bacc.Bacc:
    nc = bacc.Bacc("TRN2", target_bir_lowering=False, debug=debug,
                   num_devices=cfg.cores, num_swdge_queues=4)
    f32 = mybir.dt.float32
    bf16 = mybir.dt.bfloat16
    fp8 = mybir.dt.float8e4
    i16 = mybir.dt.int16
    i32 = mybir.dt.int32

    H, F1, C, HF, ZW, FW = cfg.H, cfg.F1, cfg.C, cfg.HF, cfg.ZW, cfg.FW
    NB, CM, CMH, WCAP = cfg.NB, cfg.CM, cfg.CMH, cfg.WCAP
    HALF, NPAD, NCH, NBT = cfg.HALF, cfg.NPAD, cfg.NCH, cfg.NBT
    NCORE = cfg.NCORE
    HBLK = NBT // 2            # blocks per half table (216)

    # ---- I/O ----
    xT_d = nc.dram_tensor("xT", [cfg.IN, NPAD], bf16, kind="ExternalInput")
    Waug_d = nc.dram_tensor("Waug", [cfg.IN, ZW], bf16, kind="ExternalInput")
    W2aug_d = nc.dram_tensor("W2aug", [HF, C + 2], f32, kind="ExternalInput")
    b1t_d = nc.dram_tensor("b1t", [P, HF], f32, kind="ExternalInput")
    b2t_d = nc.dram_tensor("b2t", [P, C], f32, kind="ExternalInput")
    sidx_d = nc.dram_tensor("sidx", [P, NB * 2 * (WCAP // 16)], i16,
                            kind="ExternalInput")
    G8_d = nc.dram_tensor("G8", [P, NCH * P], fp8, kind="ExternalInput")
    GT8_d = nc.dram_tensor("GT8", [P, NCH * P], fp8, kind="ExternalInput")
    nid0_d = nc.dram_tensor("nid0", [P, NB], i32, kind="ExternalInput")
    out_d = nc.dram_tensor("out", [NCORE, C], f32, kind="ExternalOutput")

    dk = dict(kind="ExternalOutput") if cfg.debug_outs else {}
    z_lo_d = nc.dram_tensor("z_lo", [HALF, FW], bf16, **dk)
    z_hi_d = nc.dram_tensor("z_hi", [HALF, FW], bf16, **dk)
    ert_d = nc.dram_tensor("ert", [P * NBT, H], bf16, **dk)    # [(p, gb), 8]
    t2loc_d = nc.dram_tensor("t2loc", [NCORE, C + 2], f32)
    t2_d = nc.dram_tensor("t2_tbl", [NPAD, C + 2], f32)
    if cfg.debug_outs:
        t2dbg_d = nc.dram_tensor("t2dbg", [NPAD, C + 2], f32, kind="ExternalOutput")
        h1dbg_d = nc.dram_tensor("h1dbg", [NCORE, HF], f32, kind="ExternalOutput")
        erowndbg_d = nc.dram_tensor("erowndbg", [P, NB, H], f32, kind="ExternalOutput")
        zseldbg_d = nc.dram_tensor("zseldbg", [P, CM, FW], bf16, kind="ExternalOutput")
        accdbg_d = nc.dram_tensor("accdbg", [P, NB, HF + H], f32, kind="ExternalOutput")
        erddbg_d = nc.dram_tensor("erddbg", [P, NB, CM, H], f32, kind="ExternalOutput")
    t2p_lo_d = nc.dram_tensor("t2p_lo", [HALF, 64], f32)
    t2p_hi_d = nc.dram_tensor("t2p_hi", [HALF, 64], f32)

    with tile.TileContext(nc) as tc:
        for _rep in range(cfg.body_reps):
            # ================= persistent tiles =================
            with tc.tile_pool(name="persist", bufs=1) as pc:
                sidx_t = pc.tile([P, NB * 2 * (WCAP // 16)], i16)
                nc.sync.dma_start(out=sidx_t[:], in_=sidx_d[:])
                b1_t = pc.tile([P, HF], f32)
                nc.sync.dma_start(out=b1_t[:], in_=b1t_d[:])
                b2_t = pc.tile([P, C], f32)
                nc.sync.dma_start(out=b2_t[:], in_=b2t_d[:])
                w2a_t = pc.tile([P, 2, C + 2], f32)
                nc.sync.dma_start(
                    out=w2a_t[:],
                    in_=W2aug_d[:].rearrange("(k p) c -> p k c", p=P))
                ident = pc.tile([P, P], f32)
                from concourse.masks import make_identity
                make_identity(nc, ident[:])
                nid_t = pc.tile([P, NB], i32)
                nc.sync.dma_start(out=nid_t[:], in_=nid0_d[:])
                z2rows = pc.tile([P, NB, C + 2], f32)   # layer-2 aug, own nodes
                er2b = pc.tile([P, NB, 1], bf16)        # er2 (own), bf16

                # ================= Phase Z =================
                with tc.tile_pool(name="zw", bufs=3) as zw, \
                     tc.tile_pool(name="zs", bufs=3) as zsp, \
                     tc.tile_pool(name="zp", bufs=4, space="PSUM") as zp:
                    waug_t = zsp.tile([P, ZW], bf16, tag="waug")
                    nc.sync.dma_start(out=waug_t[:], in_=Waug_d[:])
                    XG = 8
                    for g0 in range(NBT // XG):
                        b0 = g0 * XG
                        xt_t = zw.tile([P, XG * P], bf16, tag="xt")
                        nc.sync.dma_start(
                            out=xt_t[:], in_=xT_d[:, b0 * P:(b0 + XG) * P])
                        zb = zw.tile([P, XG, ZW], bf16, tag="zb")
                        for j in range(XG):
                            gb = b0 + j
                            pz = zp.tile([P, ZW], f32, tag="pz", space="PSUM")
                            nc.tensor.matmul(
                                out=pz[:], lhsT=xt_t[:, j * P:(j + 1) * P],
                                rhs=waug_t[:], start=True, stop=True)
                            if j % 2 == 0:
                                nc.vector.tensor_copy(out=zb[:, j, :], in_=pz[:])
                            else:
                                nc.scalar.copy(out=zb[:, j, :], in_=pz[:])
                        # batched stores: z rows (cols 0:272) + er block-T
                        if b0 < HBLK:
                            zdst = z_lo_d[b0 * P:(b0 + XG) * P, 0:ZW]
                        else:
                            zdst = z_hi_d[(b0 - HBLK) * P:(b0 - HBLK + XG) * P,
                                          0:ZW]
                        nc.sync.dma_start(
                            out=zdst.rearrange("(k p) c -> p k c", p=P),
                            in_=zb[:])
                        nc.sync.dma_start(
                            out=ert_d[:].rearrange(
                                "(p g) c -> p g c", g=NBT)[:, b0:b0 + XG, :],
                            in_=zb[:, :, HF + H:HF + 2 * H])

                tc.strict_bb_all_engine_barrier()

                # ================= Phase E1 =================
                with tc.tile_pool(name="e1g", bufs=2) as e1g, \
                     tc.tile_pool(name="e1h", bufs=3) as e1h, \
                     tc.tile_pool(name="e1w", bufs=2) as e1w, \
                     tc.tile_pool(name="e1p", bufs=2, space="PSUM") as e1p, \
                     tc.tile_pool(name="ep2", bufs=2, space="PSUM") as ep2, \
                     tc.tile_pool(name="tr", bufs=2, space="PSUM") as tr, \
                     tc.tile_pool(name="tp", bufs=1, space="PSUM") as tp:
                    er_own = e1h.tile([P, NB, H], bf16, tag="er_own")
                    for b0 in range(NB):
                        nc.gpsimd.indirect_dma_start(
                            out=er_own[:, b0, :], out_offset=None, in_=ert_d[:],
                            in_offset=bass.IndirectOffsetOnAxis(
                                ap=nid_t[:, b0:b0 + 1], axis=0))
                    if cfg.debug_outs:
                        eod = e1h.tile([P, NB, H], f32, tag="eod")
                        nc.vector.tensor_copy(out=eod[:], in_=er_own[:])
                        nc.sync.dma_start(out=erowndbg_d[:], in_=eod[:])

                    for b in range(NB):
                        zsel = e1g.tile([P, CM, FW], bf16, tag="zsel")
                        for half in range(2):
                            w = b * 2 + half
                            nc.gpsimd.dma_gather(
                                out_ap=zsel[:, half * CMH:(half + 1) * CMH, :],
                                in_ap=(z_lo_d[:] if half == 0 else z_hi_d[:]),
                                idxs_ap=sidx_t[:, w * (WCAP // 16):
                                               (w + 1) * (WCAP // 16)],
                                num_idxs=WCAP, num_idxs_reg=WCAP,
                                elem_size=FW, queue_num=w % 4)
                        g_t = e1h.tile([P, CM, P], fp8, tag="g")
                        nc.sync.dma_start(
                            out=g_t[:], in_=G8_d[:, b * CM * P:(b + 1) * CM * P])
                        gt_t = e1h.tile([P, CM, P], fp8, tag="gt")
                        nc.sync.dma_start(
                            out=gt_t[:], in_=GT8_d[:, b * CM * P:(b + 1) * CM * P])

                        # er_d per edge via GT matmuls
                        erd = ep2.tile([P, CM, H], f32, tag="erd", space="PSUM")
                        for cc in range(CM):
                            nc.tensor.matmul(
                                out=erd[:, cc, :], lhsT=gt_t[:, cc, :],
                                rhs=er_own[:, b, :], start=True, stop=True)
                        # e = el_s + er_d ; leaky; exp
                        rhs = e1w.tile([P, CM, HF + H], bf16, tag="rhs")
                        ee = e1w.tile([P, CM, H], bf16, tag="ee")
                        nc.vector.tensor_tensor(
                            out=ee[:], in0=zsel[:, :, HF:HF + H], in1=erd[:],
                            op=mybir.AluOpType.add)
                        ee2 = e1w.tile([P, CM, H], bf16, tag="ee2")
                        nc.vector.tensor_scalar_mul(
                            out=ee2[:], in0=ee[:], scalar1=cfg.neg_slope)
                        nc.vector.tensor_tensor(
                            out=ee[:], in0=ee2[:], in1=ee[:],
                            op=mybir.AluOpType.max)
                        nc.scalar.activation(
                            out=rhs[:, :, HF:HF + H], in_=ee[:],
                            func=mybir.ActivationFunctionType.Exp)
                        # msg = z_s * ex
                        nc.vector.tensor_tensor(
                            out=rhs[:, :, 0:HF].rearrange(
                                "p c (h f) -> p c h f", f=F1),
                            in0=zsel[:, :, 0:HF].rearrange(
                                "p c (h f) -> p c h f", f=F1),
                            in1=rhs[:, :, HF:HF + H][:, :, :, None].to_broadcast(
                                [P, CM, H, F1]),
                            op=mybir.AluOpType.mult)
                        # aggregate
                        acc = e1p.tile([P, HF + H], f32, tag="acc", space="PSUM")
                        for cc in range(CM):
                            nc.tensor.matmul(
                                out=acc[:], lhsT=g_t[:, cc, :],
                                rhs=rhs[:, cc, :],
                                start=(cc == 0), stop=(cc == CM - 1))
                        # normalize + bias + ELU -> h1
                        den = e1w.tile([P, H], f32, tag="den")
                        nc.vector.tensor_scalar_max(
                            out=den[:], in0=acc[:, HF:HF + H], scalar1=1e-30)
                        rec = e1w.tile([P, H], f32, tag="rec")
                        nc.vector.reciprocal(out=rec[:], in_=den[:])
                        rst = e1w.tile([P, HF], f32, tag="rst")
                        nc.vector.tensor_tensor(
                            out=rst[:].rearrange("p (h f) -> p h f", f=F1),
                            in0=acc[:, 0:HF].rearrange("p (h f) -> p h f", f=F1),
                            in1=rec[:, :, None].to_broadcast([P, H, F1]),
                            op=mybir.AluOpType.mult)
                        nc.vector.tensor_tensor(
                            out=rst[:], in0=rst[:], in1=b1_t[:],
                            op=mybir.AluOpType.add)
                        h1e = e1w.tile([P, HF], f32, tag="h1e")
                        nc.scalar.activation(
                            out=h1e[:], in_=rst[:],
                            func=mybir.ActivationFunctionType.Exp)
                        nc.vector.tensor_scalar(
                            out=h1e[:], in0=h1e[:], scalar1=1.0, scalar2=0.0,
                            op0=mybir.AluOpType.subtract, op1=mybir.AluOpType.min)
                        h1 = e1w.tile([P, HF], f32, tag="h1")
                        nc.vector.tensor_scalar_max(
                            out=h1[:], in0=rst[:], scalar1=0.0)
                        nc.vector.tensor_tensor(
                            out=h1[:], in0=h1[:], in1=h1e[:],
                            op=mybir.AluOpType.add)
                        if cfg.debug_outs:
                            nc.sync.dma_start(
                                out=h1dbg_d[b * P:(b + 1) * P, :], in_=h1[:])
                            accs = e1w.tile([P, HF + H], f32, tag="accs")
                            nc.vector.tensor_copy(out=accs[:], in_=acc[:])
                            nc.sync.dma_start(out=accdbg_d[:, b, :], in_=accs[:])
                            erds = e1w.tile([P, CM, H], f32, tag="erds")
                            nc.vector.tensor_copy(out=erds[:], in_=erd[:])
                            nc.sync.dma_start(out=erddbg_d[:, b, :, :], in_=erds[:])
                            if b == 0:
                                nc.sync.dma_start(out=zseldbg_d[:], in_=zsel[:])
                        # z2aug = h1 @ W2aug via PE transposes
                        h1T = e1w.tile([P, 2, P], f32, tag="h1T")
                        for k in range(2):
                            ps_t = tr.tile([P, P], f32, tag="pst", space="PSUM")
                            nc.tensor.transpose(
                                out=ps_t[:], in_=h1[:, k * P:(k + 1) * P],
                                identity=ident[:])
                            nc.scalar.copy(out=h1T[:, k, :], in_=ps_t[:])
                        pz2 = tp.tile([C + 2, P], f32, tag="pz2", space="PSUM")
                        for k in range(2):
                            nc.tensor.matmul(
                                out=pz2[:], lhsT=w2a_t[:, k, :],
                                rhs=h1T[:, k, :], start=(k == 0), stop=(k == 1))
                        z2s = e1w.tile([C + 2, P], f32, tag="z2s")
                        nc.vector.tensor_copy(out=z2s[:], in_=pz2[:])
                        pzT = tp.tile([P, C + 2], f32, tag="pzT", space="PSUM")
                        nc.tensor.transpose(
                            out=pzT[:], in_=z2s[:],
                            identity=ident[:C + 2, :C + 2])
                        nc.scalar.copy(out=z2rows[:, b, :], in_=pzT[:])
                    # store t2loc rows; cast er2 to bf16
                    nc.vector.tensor_copy(
                        out=er2b[:], in_=z2rows[:, :, C + 1:C + 2])
                    nc.sync.dma_start(
                        out=t2loc_d[:].rearrange("(b p) c -> p b c", p=P),
                        in_=z2rows[:])

                # ================= AllGather =================
                tc.strict_bb_all_engine_barrier()
                nc.gpsimd.collective_compute(
                    "AllGather", mybir.AluOpType.bypass,
                    replica_groups=[list(range(cfg.cores))],
                    ins=[t2loc_d[:].opt()], outs=[t2_d[:].opt()])
                tc.strict_bb_all_engine_barrier()

                # expand t2 -> padded gather tables (via SBUF)
                with tc.tile_pool(name="exp", bufs=2) as xp:
                    if cfg.debug_outs:
                        t2c = xp.tile([P, NPAD // P, C + 2], f32, tag="t2c")
                        nc.sync.dma_start(
                            out=t2c[:],
                            in_=t2_d[:].rearrange("(p k) c -> p k c", p=P))
                        nc.sync.dma_start(
                            out=t2dbg_d[:].rearrange("(p k) c -> p k c", p=P),
                            in_=t2c[:])
                    for half in range(2):
                        t2a = xp.tile([P, HBLK, C + 2], f32, tag="t2a")
                        src_rows = t2_d[half * HALF:(half + 1) * HALF, :]
                        nc.sync.dma_start(
                            out=t2a[:],
                            in_=src_rows.rearrange("(b p) c -> p b c", p=P))
                        dst = (t2p_lo_d if half == 0 else t2p_hi_d)
                        nc.sync.dma_start(
                            out=dst[:, 0:C + 2].rearrange(
                                "(b p) c -> p b c", p=P),
                            in_=t2a[:])

                tc.strict_bb_all_engine_barrier()

                # ================= Phase E2 =================
                with tc.tile_pool(name="e2g", bufs=2) as e2g, \
                     tc.tile_pool(name="e2h", bufs=3) as e2h, \
                     tc.tile_pool(name="e2w", bufs=2) as e2w, \
                     tc.tile_pool(name="e2p", bufs=2, space="PSUM") as e2p, \
                     tc.tile_pool(name="eq2", bufs=2, space="PSUM") as eq2:
                    outN = e2w.tile([P, NB, C], f32, tag="outN")
                    for b in range(NB):
                        t2sel = e2g.tile([P, CM, 64], f32, tag="t2sel")
                        for half in range(2):
                            w = b * 2 + half
                            nc.gpsimd.dma_gather(
                                out_ap=t2sel[:, half * CMH:(half + 1) * CMH, :],
                                in_ap=(t2p_lo_d[:] if half == 0 else
                                       t2p_hi_d[:]),
                                idxs_ap=sidx_t[:, w * (WCAP // 16):
                                               (w + 1) * (WCAP // 16)],
                                num_idxs=WCAP, num_idxs_reg=WCAP,
                                elem_size=64, queue_num=w % 4)
                        g2_t = e2h.tile([P, CM, P], fp8, tag="g2")
                        nc.sync.dma_start(
                            out=g2_t[:],
                            in_=G8_d[:, b * CM * P:(b + 1) * CM * P])
                        gt2_t = e2h.tile([P, CM, P], fp8, tag="gt2")
                        nc.sync.dma_start(
                            out=gt2_t[:],
                            in_=GT8_d[:, b * CM * P:(b + 1) * CM * P])
                        er2d = eq2.tile([P, CM, 1], f32, tag="er2d",
                                        space="PSUM")
                        for cc in range(CM):
                            nc.tensor.matmul(
                                out=er2d[:, cc, :], lhsT=gt2_t[:, cc, :],
                                rhs=er2b[:, b, :], start=True, stop=True)
                        e2 = e2w.tile([P, CM, 1], f32, tag="e2")
                        nc.vector.tensor_tensor(
                            out=e2[:], in0=t2sel[:, :, C:C + 1], in1=er2d[:],
                            op=mybir.AluOpType.add)
                        e2b_ = e2w.tile([P, CM, 1], f32, tag="e2b")
                        nc.vector.tensor_scalar_mul(
                            out=e2b_[:], in0=e2[:], scalar1=cfg.neg_slope)
                        nc.vector.tensor_tensor(
                            out=e2[:], in0=e2b_[:], in1=e2[:],
                            op=mybir.AluOpType.max)
                        lv = e2w.tile([P, CM, C + 1], bf16, tag="lv")
                        nc.scalar.activation(
                            out=lv[:, :, C:C + 1], in_=e2[:],
                            func=mybir.ActivationFunctionType.Exp)
                        nc.vector.tensor_tensor(
                            out=lv[:, :, 0:C], in0=t2sel[:, :, 0:C],
                            in1=lv[:, :, C:C + 1].to_broadcast([P, CM, C]),
                            op=mybir.AluOpType.mult)
                        acc2 = e2p.tile([P, C + 1], f32, tag="acc2",
                                        space="PSUM")
                        for cc in range(CM):
                            nc.tensor.matmul(
                                out=acc2[:], lhsT=g2_t[:, cc, :],
                                rhs=lv[:, cc, :],
                                start=(cc == 0), stop=(cc == CM - 1))
                        den2 = e2w.tile([P, 1], f32, tag="den2")
                        nc.vector.tensor_scalar_max(
                            out=den2[:], in0=acc2[:, C:C + 1], scalar1=1e-30)
                        rec2 = e2w.tile([P, 1], f32, tag="rec2")
                        nc.vector.reciprocal(out=rec2[:], in_=den2[:])
                        nc.vector.tensor_tensor(
                            out=outN[:, b, :], in0=acc2[:, 0:C],
                            in1=rec2[:].to_broadcast([P, C]),
                            op=mybir.AluOpType.mult)
                        nc.vector.tensor_tensor(
                            out=outN[:, b, :], in0=outN[:, b, :],
                            in1=b2_t[:], op=mybir.AluOpType.add)
                    nc.sync.dma_start(
                        out=out_d[:].rearrange("(b p) c -> p b c", p=P),
                        in_=outN[:])

    nc.compile()
    return nc


# ----------------------------------------------------------------------------
# Full pipeline
# ----------------------------------------------------------------------------

_PROGRAM_CACHE = {}


def get_program(cfg: Cfg):
    key = (cfg.N, cfg.E, cfg.cores, cfg.NB, cfg.CM, cfg.body_reps, cfg.debug_outs)
    if key not in _PROGRAM_CACHE:
        _PROGRAM_CACHE[key] = build_program(cfg)
    return _PROGRAM_CACHE[key]


def run(inputs: dict, cfg: Cfg = None, verbose=False):
    if cfg is None:
        cfg = Cfg()
    t0 = time.time()
    in_maps, meta = build_host_data(inputs, cfg)
    t1 = time.time()
    nc = get_program(cfg)
    t2 = time.time()
    res = bass_utils.run_bass_kernel_spmd(
        nc, in_maps, core_ids=list(range(cfg.cores)))
    t3 = time.time()
    if verbose:
        print(f"host prep {t1-t0:.2f}s  program {t2-t1:.2f}s  run {t3-t2:.2f}s")
    out = np.concatenate([res.results[c]["out"] for c in range(cfg.cores)],
                         axis=0)                      # [NPAD, C] by position
    return out[meta["pos"], :]                        # unpermute -> [N, C]


# ============================================================================
# Harness entry point
# ============================================================================

def kernel(**inputs) -> np.ndarray:
    """Full-input GAT forward on 8 NeuronCores. Returns [N, C] float32."""
    return run(inputs, Cfg())


def _trivial_program(cores=8):
    nc = bacc.Bacc("TRN2", target_bir_lowering=False, debug=False,
                   num_devices=cores)
    f32 = mybir.dt.float32
    x_in = nc.dram_tensor("x", [128, 64], f32, kind="ExternalInput")
    out = nc.dram_tensor("out", [128, 64], f32, kind="ExternalOutput")
    with tile.TileContext(nc) as tc:
        with tc.tile_pool(name="c", bufs=1) as cp:
            t = cp.tile([128, 64], f32)
            nc.sync.dma_start(out=t[:], in_=x_in[:])
            nc.scalar.mul(out=t[:], in_=t[:], mul=2.0)
            nc.sync.dma_start(out=out[:], in_=t[:])
    nc.compile()
    return nc


def estimate_hw_time_ns(inputs, iters=30, reps_hi=5):
    """HW time estimate via slope: min wall time of the jitted program at
    body_reps=reps_hi minus body_reps=1, divided by (reps_hi-1). Removes the
    (noisy) PJRT/axon dispatch overhead without needing a trivial-program
    floor."""
    from timing import time_program
    cfg1 = Cfg()
    in_maps, _ = build_host_data(inputs, cfg1)
    nc1 = get_program(cfg1)
    cfgR = Cfg(body_reps=reps_hi)
    ncR = get_program(cfgR)
    _, t1 = time_program(nc1, in_maps, iters=iters)
    _, tR = time_program(ncR, in_maps, iters=iters)
    per_body = (tR["min_s"] - t1["min_s"]) / (reps_hi - 1)
    print(f"  (wall min reps1 {t1['min_s']*1e3:.2f}ms, "
          f"reps{reps_hi} {tR['min_s']*1e3:.2f}ms -> "
          f"{per_body*1e3:.3f}ms/body)")
    return max(per_body, 0.0) * 1e9
